# revision 1
# baseline (speedup 1.0000x reference)
"""Trainium2 Bass kernel for a binarized 4-layer MLP (eval mode).

Reference computation (per row of x [B=16384, 784]):
  h1 = x @ sign(w1).T + b1;  s1 = sign(bn1(h1))        (clip doesn't change sign)
  h2 = s1 @ sign(w2).T + b2; s2 = sign(bn2(h2))
  h3 = s2 @ sign(w3).T + b3; y3 = clip(bn3(h3), -1, 1)
  z  = y3 @ w4.T + b4;       out = log_softmax(z)

Sharding: pure data-parallel over the batch across 8 NeuronCores
(weights replicated, no collectives).

Numerics:
  - L1 splits x on the HOST into xa = fp16(x) plus a scaled fp8 residual
    rb = fp8e4((x - xa) * 2^9).  The residual's stationary operand is
    sign(w1) * 2^-9 (exactly representable in fp8e4), so rb @ (S1*2^-9)
    accumulates the residual at its true scale into the SAME fp32 PSUM
    as the fp16 stream -- no combine step.  Total L1 error <= 2^-15|x|,
    which measures 7.6e-3 rel err end-to-end on the actual inputs
    (tolerance 2e-2; the chaotic sign-flip cascade is the amplifier).
    The fp16 stream is 7 uniform 112-row passes (784 = 7*112); the
    residual is 3 DoubleRow passes of [112, 2] row pairs (224 rows each)
    plus one plain-fp8 112-row pass -- fp8 DR does 2 rows/cycle, so the
    residual costs ~4 half-width passes instead of 7.
  - L2/L3: both operands are exactly +-1/0 in fp8e4 -> DoubleRow fp8
    matmuls produce bit-exact integer sums in fp32 PSUM.
  - BN + bias folding: bn(h + b) = A*h + C with A = g*rsqrt(v+eps),
    C = A*(b - m) + beta, applied per-partition by the Sign/Identity
    activations (fp32 internally).

Schedule notes (from NTFF trace analysis):
  - 112-row fp16 matmuls pace at the N-cycle rate (~216ns at N=512);
    128-row ones pay +40ns and 512-wide DoubleRow pays +0.1ns/col, so
    L1 uses 112-row tiles and L2/L3 stay 256 columns wide.
  - every dma_start costs ~620ns of serialized Sync-engine time to
    trigger regardless of size, so inputs/weights load as FEW large
    transfers covering all four batch groups at once; only the
    group-0 block-0 critical path keeps small fine-grained transfers.
  - a dummy-matmul burst on a zero tile warms the PE HAM clock gate
    (1.2 -> 2.4 GHz) while the startup DMAs are still in flight.
  - h1/h2/h3 activations are split into two tiles each so the next
    layer's first matmul depends only on the first half (tile-granular
    deps would stall the PE at every phase boundary).
  - the log-softmax epilogue for group g-1 hides under group g; in the
    last group it is interleaved per batch-tile behind L4.
"""

import sys

if "/opt/trn_rl_repo" not in sys.path:
    sys.path.insert(0, "/opt/trn_rl_repo")

import numpy as np

D_IN, H1, H2, H3, NCLS = 784, 3072, 1536, 768, 10
B, NCORES = 16384, 8
BC = B // NCORES          # batch rows per core
W = 512                   # batch columns per group
NG = BC // W              # groups per core
KP = 112                  # L1 k-tile partition size (784 = 7 * 112)
K1T = D_IN // KP          # 7 fp16 passes
QR = 3                    # residual DoubleRow passes ([112, 2] row pairs)
M1, M2, M3 = H1 // 128, H2 // 128, H3 // 128   # 24, 12, 6
K2P, K3P = H1 // 256, H2 // 256                # DoubleRow k-pair iters: 12, 6
K4T = H3 // 128                                # 6
NCST = 2 * M1 + 2 * M2 + 2 * M3 + NCLS         # packed per-tile consts: 94
BN_EPS = 1e-5
RSH = 9                   # residual scale: rb = fp8(r * 2^RSH), w = +-2^-RSH
NWARM = 3
NWARM2 = 20

_cached = {}


def _build(bc):
    import concourse.bacc as bacc
    import concourse.mybir as mybir
    import concourse.tile as tile

    dt = mybir.dt
    AF = mybir.ActivationFunctionType
    PM = mybir.MatmulPerfMode
    ALU = mybir.AluOpType

    ng = bc // W
    nc = bacc.Bacc("TRN2", target_bir_lowering=False, debug=False,
                   num_devices=NCORES)

    # xa = fp16(x).T per core [784, bc]; row k = feature k
    xa = nc.declare_dram_parameter("xa", [D_IN, bc], dt.float16,
                                   isOutput=False)
    # xr = fp8((x - xa) * 2^RSH).T per core [784, bc]; rows 224q..224(q+1)
    # form DoubleRow pair tile q ((e p) order), rows 672:784 the plain pass
    xr = nc.declare_dram_parameter("xr", [D_IN, bc], dt.float8e4,
                                   isOutput=False)
    w1t = nc.declare_dram_parameter("w1t", [D_IN, H1], dt.float16,
                                    isOutput=False)
    # w1r = sign(w1).T * 2^-RSH in fp8, same row scheme as xr
    w1r = nc.declare_dram_parameter("w1r", [D_IN, H1], dt.float8e4,
                                    isOutput=False)
    w2t = nc.declare_dram_parameter("w2t", [H1, H2], dt.float8e4,
                                    isOutput=False)
    w3t = nc.declare_dram_parameter("w3t", [H2, H3], dt.float8e4,
                                    isOutput=False)
    w4t = nc.declare_dram_parameter("w4t", [H3, NCLS], dt.bfloat16,
                                    isOutput=False)
    cst = nc.declare_dram_parameter("cst", [128, NCST], dt.float32,
                                    isOutput=False)
    wrm = nc.declare_dram_parameter("wrm", [128, W], dt.float16,
                                    isOutput=False)
    out = nc.declare_dram_parameter("out", [bc, NCLS], dt.float32,
                                    isOutput=True)

    with tile.TileContext(nc) as tc, \
            tc.tile_pool(name="wts", bufs=1) as wp, \
            tc.tile_pool(name="act", bufs=1) as ap_, \
            tc.tile_pool(name="eps", bufs=2) as ep, \
            tc.tile_pool(name="ps", bufs=4, space="PSUM") as ps, \
            tc.tile_pool(name="ps4", bufs=2, space="PSUM") as ps4, \
            tc.tile_pool(name="psw", bufs=1, space="PSUM") as pw:

        # ---- HAM warm-up: a short accumulate chain on a zero tile keeps
        # the PE busy while the startup DMAs stream so the clock gate
        # lifts (1.2 -> 2.4 GHz) sooner.
        wrmb = wp.tile([128, W], dt.float16, tag="wrm")
        nc.sync.dma_start(wrmb[:], wrm[:])
        pwt = pw.tile([128, W], dt.float32, tag="pw")
        for wi in range(NWARM):
            nc.tensor.matmul(pwt[:], wrmb[:, 0:128], wrmb[:],
                             start=(wi == 0), stop=(wi == NWARM - 1))

        # ---- startup-critical transfers first.  Trigger order is the
        # schedule: group-0 x and the first 512-col block of w1/w1r go
        # first (fine-grained), then one (w1, w1r) transfer per 512-col
        # block paced to the group-0 chain consumption, then the bulk.
        x00 = wp.tile([KP, W], dt.float16, tag="x00")
        nc.sync.dma_start(x00[:], xa[0:KP, 0:W])
        w1k0a = wp.tile([KP, W], dt.float16, tag="w1k0a")
        nc.sync.dma_start(w1k0a[:], w1t[0:KP, 0:W])
        x0r = wp.tile([KP, K1T - 1, W], dt.float16, tag="x0r")
        nc.sync.dma_start(x0r[:], xa[KP:D_IN, 0:W].rearrange(
            "(k p) w -> p k w", p=KP))
        w1ka = wp.tile([KP, K1T - 1, W], dt.float16, tag="w1ka")
        nc.sync.dma_start(w1ka[:], w1t[KP:D_IN, 0:W].rearrange(
            "(k p) w -> p k w", p=KP))
        cstb = wp.tile([128, NCST], dt.float32, tag="cst")
        nc.sync.dma_start(cstb[:], cst[:])
        # group-0 residual moving data: rows (2q+e)*112+p are the pair
        # tiles, rows 672:784 the plain pass -- natural feature order
        xr0 = wp.tile([KP, K1T, W], dt.float8e4, tag="xr0")
        nc.sync.dma_start(xr0[:], xr[:, 0:W].rearrange(
            "(j p) w -> p j w", p=KP))
        # w1 and residual weights per 512-col block (block 0 of w1 is the
        # two tiles above)
        wrcb = []
        t = wp.tile([KP, K1T, W], dt.float8e4, tag="wr0", name="wr0")
        nc.sync.dma_start(t[:], w1r[:, 0:W].rearrange(
            "(j p) n -> p j n", p=KP))
        wrcb.append(t)
        w1cb = [None]
        for b in range(1, M1 // 4):
            t = wp.tile([KP, K1T, W], dt.float16, tag=f"w1c{b}",
                        name=f"w1c{b}")
            nc.sync.dma_start(t[:], w1t[:, b * W:(b + 1) * W].rearrange(
                "(k p) n -> p k n", p=KP))
            w1cb.append(t)
            t = wp.tile([KP, K1T, W], dt.float8e4, tag=f"wr{b}",
                        name=f"wr{b}")
            nc.sync.dma_start(t[:], w1r[:, b * W:(b + 1) * W].rearrange(
                "(j p) n -> p j n", p=KP))
            wrcb.append(t)

        def w1v(k, mt):
            b, i = mt // 4, mt % 4
            cs = slice(i * 128, (i + 1) * 128)
            if b == 0:
                return w1k0a[:, cs] if k == 0 else w1ka[:, k - 1, cs]
            return w1cb[b][:, k, cs]

        # fp16 x and residual moving data for groups 1..3, one transfer
        xab = wp.tile([KP, K1T, (ng - 1) * W], dt.float16, tag="xab")
        nc.sync.dma_start(xab[:], xa[:, W:bc].rearrange(
            "(k p) w -> p k w", p=KP))
        xr123 = wp.tile([KP, K1T, (ng - 1) * W], dt.float8e4, tag="xr123")
        nc.sync.dma_start(xr123[:], xr[:, W:bc].rearrange(
            "(j p) w -> p j w", p=KP))

        w2sb = wp.tile([128, 2 * K2P, H2], dt.float8e4, tag="w2")
        nc.sync.dma_start(w2sb[:], w2t.ap().rearrange(
            "(t p) n -> p t n", p=128))
        w3sb = wp.tile([128, 2 * K3P, H3], dt.float8e4, tag="w3")
        nc.sync.dma_start(w3sb[:], w3t.ap().rearrange(
            "(t p) n -> p t n", p=128))
        w4sb = wp.tile([128, K4T, NCLS], dt.bfloat16, tag="w4")
        nc.sync.dma_start(w4sb[:],
                          w4t.ap().rearrange("(kt p) n -> p kt n", p=128))

        # const views into the packed per-output-tile scale/bias table
        def a1v(mt): return cstb[:, mt:mt + 1]
        def c1v(mt): return cstb[:, M1 + mt:M1 + mt + 1]
        def a2v(mt): return cstb[:, 2 * M1 + mt:2 * M1 + mt + 1]
        def c2v(mt): return cstb[:, 2 * M1 + M2 + mt:2 * M1 + M2 + mt + 1]
        def a3v(mt):
            o = 2 * M1 + 2 * M2
            return cstb[:, o + mt:o + mt + 1]
        def c3v(mt):
            o = 2 * M1 + 2 * M2 + M3
            return cstb[:, o + mt:o + mt + 1]
        b4v = cstb[:, NCST - NCLS:NCST]

        zout = wp.tile([128, ng * 4, NCLS], dt.float32, tag="zout")
        ssum = wp.tile([128, ng * 4], dt.float32, tag="ssum")
        lsum = wp.tile([128, ng * 4], dt.float32, tag="lsum")

        def emit_epilogue(lo, hi, dma=True):
            # log_softmax over the free dim; |z| is small so no max-shift
            for r in range(lo, hi):
                e = ep.tile([128, NCLS], dt.float32, tag="e")
                nc.scalar.activation(e[:], zout[:, r, :], AF.Exp,
                                     accum_out=ssum[:, r:r + 1])
            nc.scalar.activation(lsum[:, lo:hi], ssum[:, lo:hi], AF.Ln)
            for r in range(lo, hi):
                nc.vector.tensor_scalar(zout[:, r, :], zout[:, r, :],
                                        lsum[:, r:r + 1], None,
                                        op0=ALU.subtract)
            if dma:
                nc.sync.dma_start(
                    out.ap()[lo * 128:hi * 128, :].rearrange(
                        "(g p) n -> p g n", p=128),
                    zout[:, lo:hi, :])

        def l1_resid(pt, mt, g):
            # residual accumulation into pt (start=False: the fp16 stream
            # already cleared the bank); per 256-col half, 3 DoubleRow
            # pair passes + 1 plain fp8 pass
            b, i = mt // 4, mt % 4
            cs = slice(i * 128, (i + 1) * 128)
            xv = xr0 if g == 0 else xr123
            off = 0 if g == 0 else (g - 1) * W
            for h in range(2):
                hs = slice(off + h * 256, off + (h + 1) * 256)
                for q in range(QR):
                    nc.tensor.matmul(pt[:, h * 256:(h + 1) * 256],
                                     wrcb[b][:, 2 * q:2 * q + 2, cs],
                                     xv[:, 2 * q:2 * q + 2, hs],
                                     start=False, stop=False,
                                     perf_mode=PM.DoubleRow)
                nc.tensor.matmul(pt[:, h * 256:(h + 1) * 256],
                                 wrcb[b][:, 2 * QR, cs],
                                 xv[:, 2 * QR, hs],
                                 start=False, stop=True)

        def xav(k, g):
            if g == 0:
                return x00[:] if k == 0 else x0r[:, k - 1, :]
            return xab[:, k, (g - 1) * W:g * W]

        for g in range(ng):
            if g > 0:
                # epilogue for the previous group hides under this
                # group's L1 matmuls
                emit_epilogue(4 * (g - 1), 4 * g)

            # ---- L1: [784 -> 3072], 7 fp16 passes + fp8 residual
            h1t = [ap_.tile([128, K2P, W], dt.float8e4, tag="h1a",
                            name="h1a"),
                   ap_.tile([128, K2P, W], dt.float8e4, tag="h1b",
                            name="h1b")]
            for mt in range(M1):
                pt = ps.tile([128, W], dt.float32, tag="ps")
                for k in range(K1T):
                    nc.tensor.matmul(pt[:], w1v(k, mt), xav(k, g),
                                     start=(k == 0), stop=False)
                    if g == 0 and mt == 0 and k == 0:
                        # second warm burst: keeps the PE busy through the
                        # data-wait hole while the rest of the group-0
                        # transfers land, so the HAM clock gate lifts at
                        # ~13us instead of ~22us
                        for wi in range(NWARM2):
                            nc.tensor.matmul(pwt[:], wrmb[:, 0:128],
                                             wrmb[:], start=(wi == 0),
                                             stop=(wi == NWARM2 - 1))
                l1_resid(pt, mt, g)
                nc.scalar.activation(h1t[mt // 12][:, mt % 12, :], pt[:],
                                     AF.Sign, bias=c1v(mt), scale=a1v(mt))

            # ---- L2-L4 iterate over 256-column halves of the group (a
            # 512-wide DoubleRow matmul paces ~0.52 ns/col vs 0.425 at 256)
            for h in range(2):
                hs = slice(h * 256, (h + 1) * 256)
                # ---- L2: [3072 -> 1536], fp8 DoubleRow
                h2t = [ap_.tile([128, K3P, 256], dt.float8e4, tag="h2a",
                                name="h2a"),
                       ap_.tile([128, K3P, 256], dt.float8e4, tag="h2b",
                                name="h2b")]
                for mt in range(M2):
                    pt = ps.tile([128, 256], dt.float32, tag="ps")
                    for kp in range(K2P):
                        nc.tensor.matmul(
                            pt[:],
                            w2sb[:, 2 * kp:2 * kp + 2,
                                 mt * 128:(mt + 1) * 128],
                            h1t[kp // 6][:, 2 * (kp % 6):2 * (kp % 6) + 2,
                                         hs],
                            start=(kp == 0), stop=(kp == K2P - 1),
                            perf_mode=PM.DoubleRow)
                    nc.scalar.activation(h2t[mt // 6][:, mt % 6, :], pt[:],
                                         AF.Sign, bias=c2v(mt),
                                         scale=a2v(mt))

                # ---- L3: [1536 -> 768], fp8 DoubleRow; scale/bias on the
                # Scalar engine (Identity), clip on DVE; bf16 output keeps
                # L4 single-pass (fp32 moving data double-pumps the PE)
                h3t = [ap_.tile([128, K4T // 2, 256], dt.bfloat16,
                                tag="h3a", name="h3a"),
                       ap_.tile([128, K4T // 2, 256], dt.bfloat16,
                                tag="h3b", name="h3b")]
                for mt in range(M3):
                    pt = ps.tile([128, 256], dt.float32, tag="ps")
                    for kp in range(K3P):
                        nc.tensor.matmul(
                            pt[:],
                            w3sb[:, 2 * kp:2 * kp + 2,
                                 mt * 128:(mt + 1) * 128],
                            h2t[kp // 3][:, 2 * (kp % 3):2 * (kp % 3) + 2,
                                         :],
                            start=(kp == 0), stop=(kp == K3P - 1),
                            perf_mode=PM.DoubleRow)
                    h3v = h3t[mt // 3][:, mt % 3, :]
                    nc.scalar.activation(h3v, pt[:], AF.Identity,
                                         bias=c3v(mt), scale=a3v(mt))
                    nc.vector.tensor_scalar(h3v, h3v, 1.0, -1.0,
                                            op0=ALU.min, op1=ALU.max)

                # ---- L4: logits z = y3 @ w4.T + b4, [batch-tile, 10]
                for bt in range(2):
                    r = 4 * g + 2 * h + bt
                    p4 = ps4.tile([128, NCLS], dt.float32, tag="p4")
                    for kt in range(K4T):
                        nc.tensor.matmul(p4[:],
                                         h3t[kt // 3][:, kt % 3,
                                             bt * 128:(bt + 1) * 128],
                                         w4sb[:, kt, :],
                                         start=(kt == 0),
                                         stop=(kt == K4T - 1))
                    nc.vector.tensor_add(zout[:, r, :], p4[:], b4v)
                    if g == ng - 1:
                        # last group: per-tile epilogue rides behind the
                        # next batch-tile's L4 matmuls; one batched out
                        # DMA at the end (each trigger costs ~620ns of
                        # Sync time in the tail)
                        emit_epilogue(r, r + 1, dma=False)

        nc.sync.dma_start(
            out.ap()[(ng * 4 - 4) * 128:ng * 4 * 128, :].rearrange(
                "(g p) n -> p g n", p=128),
            zout[:, ng * 4 - 4:ng * 4, :])

    nc.finalize()
    return nc


def _prep(x, w1, b1, w2, b2, w3, b3, w4, b4,
          g1, be1, m1, v1, g2, be2, m2, v2, g3, be3, m3, v3):
    """Host-side prep: transposes, binarized weight casts, BN folds, and
    the fp16 + scaled-fp8 split of x (plain transposes -- the DoubleRow
    pair row order (e p) matches the natural feature order)."""
    import concourse.mybir as mybir
    f8 = mybir.dt.np(mybir.dt.float8e4)
    bf16 = mybir.dt.np(mybir.dt.bfloat16)

    def fold(g, be, m, v, b):
        a = (g / np.sqrt(v + np.float32(BN_EPS))).astype(np.float32)
        c = (a * (b - m) + be).astype(np.float32)
        return a, c

    a1, c1 = fold(g1, be1, m1, v1, b1)
    a2, c2 = fold(g2, be2, m2, v2, b2)
    a3, c3 = fold(g3, be3, m3, v3, b3)

    def cols(v, mtiles):
        return v.reshape(mtiles, 128).T

    cstm = np.zeros((128, NCST), np.float32)
    o = 0
    for v, m in ((a1, M1), (c1, M1), (a2, M2), (c2, M2), (a3, M3), (c3, M3)):
        cstm[:, o:o + m] = cols(v, m)
        o += m
    cstm[:, o:o + NCLS] = b4.astype(np.float32)[None, :]

    s1t = np.sign(w1).T.astype(np.float32)          # [784, 3072]
    pre = dict(
        w1t=np.ascontiguousarray(s1t.astype(np.float16)),
        w1r=np.ascontiguousarray(
            (s1t * np.float32(2.0 ** -RSH)).astype(f8)),
        w2t=np.ascontiguousarray(np.sign(w2).T).astype(f8),
        w3t=np.ascontiguousarray(np.sign(w3).T).astype(f8),
        w4t=np.ascontiguousarray(w4.T).astype(bf16),
        cst=cstm,
        wrm=np.zeros((128, W), np.float16),
    )

    xa16 = x.astype(np.float16)
    r = x.astype(np.float32) - xa16.astype(np.float32)
    rb = (r * np.float32(2.0 ** RSH)).astype(f8)
    return pre, xa16.T, rb.T


def run(inputs, **spmd_kwargs):
    from concourse.bass_utils import run_bass_kernel_spmd

    if "nc" not in _cached:
        _cached["nc"] = _build(BC)
    nc = _cached["nc"]

    inputs = {k: np.asarray(v) for k, v in inputs.items()}
    pre, xat, xrt = _prep(**inputs)

    in_maps = []
    for core in range(NCORES):
        m = dict(pre)
        cs = slice(core * BC, (core + 1) * BC)
        m["xa"] = np.ascontiguousarray(xat[:, cs])
        m["xr"] = np.ascontiguousarray(xrt[:, cs])
        in_maps.append(m)

    res = run_bass_kernel_spmd(nc, in_maps, list(range(NCORES)), **spmd_kwargs)
    outs = [res.results[i]["out"] for i in range(NCORES)]
    return res, np.concatenate(outs, axis=0).astype(np.float32)


def kernel(**inputs):
    return run(inputs)[1]

